# revision 28
# baseline (speedup 1.0000x reference)
"""Trainium2 Bass kernel for nn_Attention_49185965473844.

Math (per example b):
    q = x @ Wq ; k = x @ Wk ; v = x @ Wv          (x: [S, D], W*: [D, D], D=32)
    A[q,k]   = sum_s q[s,q] k[s,k]  = (Wq^T G Wk)[q,k],   G = x^T x   ([32, 32])
    scores   = softmax(A, axis=q)                 (normalize down columns)
    out[q,s] = sum_k scores[q,k] v[s,k] = (M @ x^T)[q,s], M = scores @ Wv^T

Design (vs. the fp32 baseline):
  1. fp16 end-to-end (10-bit mantissa, same as the f32r/TF32 PE mode the
     fp32 baseline used).  Measured end-to-end rel err ~8e-4 (limit 2e-2).
  2. Host-side pre-permutation of x into BOTH layouts the PE needs:
       xt[b,j,k,e]    = x[b, 2048 j + e, k]   (x^T tiles: output-matmul rhs)
       xn[b,p,t,j2,k] = x[b, 2048 j2 + 128 t + p, k]  (natural tiles: Gram)
     so there are NO on-chip transposes, and every DMA (loads and stores)
     is fully contiguous in HBM with 4 KiB per partition line.
  3. The 32x32 chain (fold -> t2 -> A^T -> softmax -> M) is batched over
     PAIRS of examples on 64 partitions (block-diagonal Wk constant), since
     these ops are fixed-overhead dominated.
  4. Schedule keeps the PE stream dense: a 4-gram prologue (~64 matmuls
     back-to-back) trips the HAM clock gate to 2.4 GHz early; chains,
     M-prep and output blocks interleave with grams afterwards so every
     cross-engine hop has >=1 PE block of slack.

Sharding: pure data parallel over batch B=64 -> 8 examples per NeuronCore.
"""

import numpy as np

import concourse.bass as bass
import concourse.bacc as bacc
import concourse.tile as tile
from concourse import mybir
from concourse.bass_utils import run_bass_kernel_spmd

N_CORES = 8
B, S, D = 64, 8192, 32
PER_CORE = B // N_CORES  # 8

F32 = mybir.dt.float32
F16 = mybir.dt.float16

EB = S // 4  # 2048: e-range per j-block of the transposed layout


def build_nc(n_ex=PER_CORE, seq=S):
    """Build the per-core Bass program. Same program runs on all 8 cores."""
    assert seq == S and n_ex % 2 == 0
    n_slab = seq // 512  # 16 Gram slabs per example

    nc = bacc.Bacc("TRN2", target_bir_lowering=False, debug=False)
    xt_t = nc.declare_dram_parameter("xt", [n_ex, 4, D, EB], F16, isOutput=False)
    xn_t = nc.declare_dram_parameter("xn", [n_ex, 128, 2048], F16, isOutput=False)
    c32_t = nc.declare_dram_parameter("c32", [128, 224], F32, isOutput=False)
    c16_t = nc.declare_dram_parameter("c16", [128, 128], F16, isOutput=False)
    out_t = nc.declare_dram_parameter("out", [n_ex, 4, D, EB], F16, isOutput=True)

    with tile.TileContext(nc) as tc:
        with (
            tc.tile_pool(name="consts", bufs=1) as consts,
            tc.tile_pool(name="xt_pool", bufs=n_ex) as xt_pool,
            tc.tile_pool(name="xn_pool", bufs=n_ex) as xn_pool,
            tc.tile_pool(name="osb_pool", bufs=5) as osb_pool,
            tc.tile_pool(name="small_pool", bufs=3) as small_pool,
            tc.tile_pool(name="acc_psum", bufs=4, space="PSUM") as acc_psum,
            tc.tile_pool(name="o_psum", bufs=4, space="PSUM") as o_psum,
        ):
            # ---- constants: one fp32 DMA + one fp16 DMA ----
            c32 = consts.tile([128, 224], F32)
            nc.sync.dma_start(out=c32, in_=c32_t[:, :])
            wq4 = c32[:, 0:32]            # np.tile(Wq, (4, 1))
            wkbd = c32[0:64, 32:96]       # block-diag{Wk, Wk} on 64 parts
            wvt2 = c32[0:64, 96:224]      # wvt2[(x k), 32j+d] = Wv[d, k]
            c16 = consts.tile([128, 128], F16)
            nc.sync.dma_start(out=c16, in_=c16_t[:, :])
            blkmask = c16[:, 0:128]       # [p, c] = 1.0 iff p//32 == c//32

            def load_xn(b):
                xn = xn_pool.tile([128, 2048], F16, tag="xn", name=f"xn_{b}")
                nc.sync.dma_start(out=xn, in_=xn_t[b])
                return xn

            def load_xt(b):
                xt = xt_pool.tile([128, EB], F16, tag="xt", name=f"xt_{b}")
                nc.sync.dma_start(
                    out=xt, in_=xt_t[b].rearrange("j k e -> (j k) e")
                )
                return xt

            gram_c2s = {}  # pair index -> [128, 64] tile

            def emit_gram(b, xn):
                """16 Gram matmuls for example b; column-align the 4
                diagonal 32x32 blocks into its pair's gram_c2 columns."""
                p, x = divmod(b, 2)
                if x == 0:
                    gram_c2s[p] = small_pool.tile([128, 2 * D], F32,
                                                  tag="gram_c2",
                                                  name=f"gram_c2_{p}")
                gram_ps = acc_psum.tile([128, 128], F32, tag="acc")
                for t in range(n_slab):
                    nc.tensor.matmul(
                        gram_ps,
                        lhsT=xn[:, 128 * t:128 * (t + 1)],
                        rhs=xn[:, 128 * t:128 * (t + 1)],
                        start=(t == 0),
                        stop=(t == n_slab - 1),
                    )
                gram_c2 = gram_c2s[p]
                for j2 in range(4):
                    if j2 % 2 == 0:
                        nc.scalar.copy(
                            out=gram_c2[32 * j2:32 * (j2 + 1),
                                        32 * x:32 * (x + 1)],
                            in_=gram_ps[32 * j2:32 * (j2 + 1),
                                        32 * j2:32 * (j2 + 1)])
                    else:
                        nc.vector.tensor_copy(
                            out=gram_c2[32 * j2:32 * (j2 + 1),
                                        32 * x:32 * (x + 1)],
                            in_=gram_ps[32 * j2:32 * (j2 + 1),
                                        32 * j2:32 * (j2 + 1)])

            def emit_chain(p):
                """Paired chain: t2 = G Wq (fold over j2 + multiply), then
                A^T for both examples stacked on 64 partitions, softmax
                over q (the free dim) -> sc_sb2 [64, 32]."""
                gram_c2 = gram_c2s.pop(p)
                t2_ps = acc_psum.tile([2 * D, D], F32, tag="acc")
                nc.tensor.matmul(t2_ps, lhsT=gram_c2, rhs=wq4)
                t2_sb = small_pool.tile([2 * D, D], F32, tag="t2_sb")
                nc.scalar.copy(out=t2_sb, in_=t2_ps)
                at_ps = acc_psum.tile([2 * D, D], F32, tag="acc")
                nc.tensor.matmul(at_ps, lhsT=wkbd, rhs=t2_sb)

                nmax = small_pool.tile([2 * D, 1], F32, tag="nmax")
                nc.vector.reduce_max(
                    out=nmax, in_=at_ps, axis=mybir.AxisListType.X, negate=True
                )
                e_sb = small_pool.tile([2 * D, D], F32, tag="e_sb")
                rsum = small_pool.tile([2 * D, 1], F32, tag="rsum")
                nc.scalar.activation(
                    out=e_sb, in_=at_ps,
                    func=mybir.ActivationFunctionType.Exp,
                    bias=nmax, scale=1.0,
                    accum_out=rsum,
                )
                rinv = small_pool.tile([2 * D, 1], F32, tag="rinv")
                nc.vector.reciprocal(out=rinv, in_=rsum)
                sc_sb = small_pool.tile([2 * D, D], F32, tag="sc_sb")
                nc.vector.tensor_scalar_mul(out=sc_sb, in0=e_sb, scalar1=rinv)
                return sc_sb

            def emit_m4bd(p, sc_sb2):
                """M^T for both examples of the pair, replicated on 4
                partition blocks -> two block-diag fp16 tiles."""
                # two matmuls with different base partitions may not share a
                # PSUM tile (HW fault observed) -- use one tile per example
                m4_sb = small_pool.tile([128, 2 * D], F16, tag="m4_sb")
                for x in range(2):
                    m4_ps = acc_psum.tile([128, D], F32, tag="acc")
                    nc.tensor.matmul(
                        m4_ps,
                        lhsT=wvt2[32 * x:32 * (x + 1), :],
                        rhs=sc_sb2[32 * x:32 * (x + 1), :],
                    )
                    nc.scalar.copy(out=m4_sb[:, 32 * x:32 * (x + 1)],
                                   in_=m4_ps)
                bds = []
                for x in range(2):
                    bd = small_pool.tile([128, 128], F16, tag="bd",
                                         name=f"bd_{p}_{x}")
                    sl = m4_sb[:, 32 * x:32 * (x + 1)]
                    m4_bcast = bass.AP(
                        tensor=sl.tensor,
                        offset=sl.offset,
                        ap=[list(sl.ap[0]), [0, 4], list(sl.ap[1])],
                    )
                    nc.gpsimd.tensor_mul(
                        out=bd.rearrange("p (r q) -> p r q", r=4),
                        in0=m4_bcast,
                        in1=blkmask.rearrange("p (r q) -> p r q", r=4),
                    )
                    bds.append(bd)
                return bds

            def emit_out(b, xt, bd):
                """4 block-diagonal matmuls against resident x^T; each
                512-col quarter is cast and stored as soon as it's ready
                (1 KiB per partition line), overlapping store with cast.
                Stores ride the sync ring: a dma_start blocks its issuing
                engine's queue when the HWDGE ring backs up, and sync has
                nothing else to do after the upfront loads (scalar carries
                PE-critical copies)."""
                osb = osb_pool.tile([128, 2048], F16, tag="osb",
                                    name=f"osb_{b}")
                dst = out_t[b].rearrange("j q e -> (j q) e")
                for g in range(4):
                    o_ps = o_psum.tile([128, 512], F32, tag="o")
                    nc.tensor.matmul(
                        o_ps, lhsT=bd, rhs=xt[:, 512 * g:512 * (g + 1)],
                    )
                    if g % 2 == 0:
                        nc.scalar.copy(
                            out=osb[:, 512 * g:512 * (g + 1)], in_=o_ps)
                    else:
                        nc.vector.tensor_copy(
                            out=osb[:, 512 * g:512 * (g + 1)], in_=o_ps)
                    nc.sync.dma_start(
                        out=dst[:, 512 * g:512 * (g + 1)],
                        in_=osb[:, 512 * g:512 * (g + 1)],
                    )

            # All loads queued upfront; xn tiles first (gram needs them
            # immediately, xt only at the out stage).
            xns = [load_xn(b) for b in range(n_ex)]
            xts = [load_xt(b) for b in range(n_ex)]

            # Interleaved pair-pipelined schedule: gram blocks (PE-heavy)
            # alternate with chain/M-prep/output blocks (ACT/DVE-heavy) all
            # the way through, so the PE stream stays dense (HAM stays
            # warm) and the PSUM->SBUF casts overlap gram matmuls.  Every
            # consumer sits >=1 PE block after its producer.
            assert n_ex == 8
            sched = ["G0", "G1", "G2", "G3", "C0", "G4", "C1", "G5", "M0",
                     "G6", "C2", "G7", "O0", "M1", "O1", "C3", "O2", "M2",
                     "O3", "O4", "M3", "O5", "O6", "O7"]
            scs, bds = {}, {}
            for op in sched:
                kind, idx = op[0], int(op[1:])
                if kind == "G":
                    emit_gram(idx, xns[idx])
                elif kind == "C":
                    scs[idx] = emit_chain(idx)
                elif kind == "M":
                    bds[idx] = emit_m4bd(idx, scs.pop(idx))
                else:
                    emit_out(idx, xts[idx], bds[idx // 2][idx % 2])

    nc.compile()
    return nc


_CACHED_NC = None


def _get_nc():
    global _CACHED_NC
    if _CACHED_NC is None:
        _CACHED_NC = build_nc()
    return _CACHED_NC


def make_consts(wq, wk, wv):
    """c32 [128, 224] fp32: Wq tiled 4x down | blockdiag{Wk,Wk} | Wv^T
    tiled 4x along cols and 2x down.  c16 [128, 128] fp16: block mask."""
    c32 = np.zeros((128, 224), dtype=np.float32)
    c32[:, 0:32] = np.tile(wq, (4, 1))
    c32[0:D, 32:64] = wk
    c32[D:2 * D, 64:96] = wk
    c32[0:2 * D, 96:224] = np.tile(wv.T, (2, 4))
    blk = np.arange(128) // 32
    c16 = (blk[:, None] == blk[None, :]).astype(np.float16)
    return c32, c16


def make_xt(x16):
    """[n, 8192, 32] fp16 -> [n, 4, 32, 2048] with
    xt[b, j, k, e] = x[b, 2048 j + e, k]."""
    n = x16.shape[0]
    return np.ascontiguousarray(
        x16.reshape(n, 4, EB, D).transpose(0, 1, 3, 2)
    )


def make_xn(x16):
    """[n, 8192, 32] fp16 -> [n, 128, 2048] with
    xn[b, p, (t, j2, k)] = x[b, 2048 j2 + 128 t + p, k]."""
    n = x16.shape[0]
    return np.ascontiguousarray(
        x16.reshape(n, 4, 16, 128, D).transpose(0, 3, 2, 1, 4)
    ).reshape(n, 128, 2048)


def unpack_out(res_out):
    """[n, 4, 32, 2048] fp16 -> [n, 32, 8192] fp32."""
    return np.ascontiguousarray(
        res_out.astype(np.float32).transpose(0, 2, 1, 3)
    ).reshape(res_out.shape[0], D, S)


def make_in_maps(x, wq, wk, wv):
    c32, c16 = make_consts(wq, wk, wv)
    x16 = x.astype(np.float16)
    return [
        {
            "xt": make_xt(x16[c * PER_CORE:(c + 1) * PER_CORE]),
            "xn": make_xn(x16[c * PER_CORE:(c + 1) * PER_CORE]),
            "c32": c32,
            "c16": c16,
        }
        for c in range(N_CORES)
    ]


def kernel(x, Wq, Wk, Wv):
    x = np.asarray(x, dtype=np.float32)
    wq = np.asarray(Wq, dtype=np.float32).reshape(D, D)
    wk = np.asarray(Wk, dtype=np.float32).reshape(D, D)
    wv = np.asarray(Wv, dtype=np.float32).reshape(D, D)
    assert x.shape == (B, S, D)

    nc = _get_nc()
    in_maps = make_in_maps(x, wq, wk, wv)
    res = run_bass_kernel_spmd(nc, in_maps, list(range(N_CORES)))
    out = np.concatenate(
        [unpack_out(res.results[c]["out"]) for c in range(N_CORES)], axis=0
    )
    return out


# revision 30
# speedup vs baseline: 1.2111x; 1.2111x over previous
"""Trainium2 Bass kernel for nn_Attention_49185965473844.

Math (per example b):
    q = x @ Wq ; k = x @ Wk ; v = x @ Wv          (x: [S, D], W*: [D, D], D=32)
    A[q,k]   = sum_s q[s,q] k[s,k]  = (Wq^T G Wk)[q,k],   G = x^T x   ([32, 32])
    scores   = softmax(A, axis=q)                 (normalize down columns)
    out[q,s] = sum_k scores[q,k] v[s,k] = (M @ x^T)[q,s], M = scores @ Wv^T

Design (vs. the fp32 baseline):
  1. fp16 end-to-end (10-bit mantissa, same as the f32r/TF32 PE mode the
     fp32 baseline used).  Measured end-to-end rel err ~8e-4 (limit 2e-2).
  2. Host-side pre-permutation of x into BOTH layouts the PE needs:
       xt[b,j,k,e]    = x[b, 2048 j + e, k]   (x^T tiles: output-matmul rhs)
       xn[b,p,t,j2,k] = x[b, 2048 j2 + 128 t + p, k]  (natural tiles: Gram)
     so there are NO on-chip transposes, and every DMA (loads and stores)
     is fully contiguous in HBM with 4 KiB per partition line.
  3. The 32x32 chain (fold -> t2 -> A^T -> softmax -> M) is batched over
     PAIRS of examples on 64 partitions (block-diagonal Wk constant), since
     these ops are fixed-overhead dominated.
  4. Schedule keeps the PE stream dense: a 4-gram prologue (~64 matmuls
     back-to-back) trips the HAM clock gate to 2.4 GHz early; chains,
     M-prep and output blocks interleave with grams afterwards so every
     cross-engine hop has >=1 PE block of slack.

Sharding: pure data parallel over batch B=64 -> 8 examples per NeuronCore.
"""

import numpy as np

import concourse.bass as bass
import concourse.bacc as bacc
import concourse.tile as tile
from concourse import mybir
from concourse.bass_utils import run_bass_kernel_spmd

N_CORES = 8
B, S, D = 64, 8192, 32
PER_CORE = B // N_CORES  # 8

F32 = mybir.dt.float32
F16 = mybir.dt.float16

EB = S // 4  # 2048: e-range per j-block of the transposed layout


def build_nc(n_ex=PER_CORE, seq=S):
    """Build the per-core Bass program. Same program runs on all 8 cores."""
    assert seq == S and n_ex % 2 == 0
    n_slab = seq // 512  # 16 Gram slabs per example

    nc = bacc.Bacc("TRN2", target_bir_lowering=False, debug=False)
    xt_t = nc.declare_dram_parameter("xt", [n_ex, 4, D, EB], F16, isOutput=False)
    xn_t = nc.declare_dram_parameter("xn", [n_ex, 128, 2048], F16, isOutput=False)
    c32_t = nc.declare_dram_parameter("c32", [128, 224], F32, isOutput=False)
    c16_t = nc.declare_dram_parameter("c16", [128, 128], F16, isOutput=False)
    out_t = nc.declare_dram_parameter("out", [n_ex, 4, D, EB], F16, isOutput=True)

    with tile.TileContext(nc) as tc:
        with (
            tc.tile_pool(name="consts", bufs=1) as consts,
            tc.tile_pool(name="xt_pool", bufs=n_ex) as xt_pool,
            tc.tile_pool(name="xn_pool", bufs=n_ex) as xn_pool,
            tc.tile_pool(name="osb_pool", bufs=5) as osb_pool,
            tc.tile_pool(name="small_pool", bufs=3) as small_pool,
            tc.tile_pool(name="acc_psum", bufs=4, space="PSUM") as acc_psum,
            tc.tile_pool(name="o_psum", bufs=4, space="PSUM") as o_psum,
        ):
            # ---- constants: one fp32 DMA + one fp16 DMA ----
            c32 = consts.tile([128, 224], F32)
            nc.sync.dma_start(out=c32, in_=c32_t[:, :])
            wq4 = c32[:, 0:32]            # np.tile(Wq, (4, 1))
            wkbd = c32[0:64, 32:96]       # block-diag{Wk, Wk} on 64 parts
            wvt2 = c32[0:64, 96:224]      # wvt2[(x k), 32j+d] = Wv[d, k]
            c16 = consts.tile([128, 128], F16)
            nc.sync.dma_start(out=c16, in_=c16_t[:, :])
            blkmask = c16[:, 0:128]       # [p, c] = 1.0 iff p//32 == c//32

            def load_xn(b):
                xn = xn_pool.tile([128, 2048], F16, tag="xn", name=f"xn_{b}")
                nc.sync.dma_start(out=xn, in_=xn_t[b])
                return xn

            def load_xt(b):
                xt = xt_pool.tile([128, EB], F16, tag="xt", name=f"xt_{b}")
                nc.sync.dma_start(
                    out=xt, in_=xt_t[b].rearrange("j k e -> (j k) e")
                )
                return xt

            gram_c2s = {}  # pair index -> [128, 64] tile

            def emit_gram(b, xn):
                """16 Gram matmuls for example b; column-align the 4
                diagonal 32x32 blocks into its pair's gram_c2 columns."""
                p, x = divmod(b, 2)
                if x == 0:
                    gram_c2s[p] = small_pool.tile([128, 2 * D], F32,
                                                  tag="gram_c2",
                                                  name=f"gram_c2_{p}")
                gram_ps = acc_psum.tile([128, 128], F32, tag="acc")
                for t in range(n_slab):
                    nc.tensor.matmul(
                        gram_ps,
                        lhsT=xn[:, 128 * t:128 * (t + 1)],
                        rhs=xn[:, 128 * t:128 * (t + 1)],
                        start=(t == 0),
                        stop=(t == n_slab - 1),
                    )
                gram_c2 = gram_c2s[p]
                for j2 in range(4):
                    if j2 % 2 == 0:
                        nc.scalar.copy(
                            out=gram_c2[32 * j2:32 * (j2 + 1),
                                        32 * x:32 * (x + 1)],
                            in_=gram_ps[32 * j2:32 * (j2 + 1),
                                        32 * j2:32 * (j2 + 1)])
                    else:
                        nc.vector.tensor_copy(
                            out=gram_c2[32 * j2:32 * (j2 + 1),
                                        32 * x:32 * (x + 1)],
                            in_=gram_ps[32 * j2:32 * (j2 + 1),
                                        32 * j2:32 * (j2 + 1)])

            def emit_chain(p):
                """Paired chain: t2 = G Wq (fold over j2 + multiply), then
                A^T for both examples stacked on 64 partitions, softmax
                over q (the free dim) -> sc_sb2 [64, 32]."""
                gram_c2 = gram_c2s.pop(p)
                t2_ps = acc_psum.tile([2 * D, D], F32, tag="acc")
                nc.tensor.matmul(t2_ps, lhsT=gram_c2, rhs=wq4)
                t2_sb = small_pool.tile([2 * D, D], F32, tag="t2_sb")
                nc.scalar.copy(out=t2_sb, in_=t2_ps)
                at_ps = acc_psum.tile([2 * D, D], F32, tag="acc")
                nc.tensor.matmul(at_ps, lhsT=wkbd, rhs=t2_sb)

                nmax = small_pool.tile([2 * D, 1], F32, tag="nmax")
                nc.vector.reduce_max(
                    out=nmax, in_=at_ps, axis=mybir.AxisListType.X, negate=True
                )
                e_sb = small_pool.tile([2 * D, D], F32, tag="e_sb")
                rsum = small_pool.tile([2 * D, 1], F32, tag="rsum")
                nc.scalar.activation(
                    out=e_sb, in_=at_ps,
                    func=mybir.ActivationFunctionType.Exp,
                    bias=nmax, scale=1.0,
                    accum_out=rsum,
                )
                rinv = small_pool.tile([2 * D, 1], F32, tag="rinv")
                nc.vector.reciprocal(out=rinv, in_=rsum)
                sc_sb = small_pool.tile([2 * D, D], F32, tag="sc_sb")
                nc.vector.tensor_scalar_mul(out=sc_sb, in0=e_sb, scalar1=rinv)
                return sc_sb

            def emit_m4bd(p, sc_sb2):
                """M^T for both examples of the pair, replicated on 4
                partition blocks -> two block-diag fp16 tiles."""
                # two matmuls with different base partitions may not share a
                # PSUM tile (HW fault observed) -- use one tile per example
                m4_sb = small_pool.tile([128, 2 * D], F16, tag="m4_sb")
                for x in range(2):
                    m4_ps = acc_psum.tile([128, D], F32, tag="acc")
                    nc.tensor.matmul(
                        m4_ps,
                        lhsT=wvt2[32 * x:32 * (x + 1), :],
                        rhs=sc_sb2[32 * x:32 * (x + 1), :],
                    )
                    nc.scalar.copy(out=m4_sb[:, 32 * x:32 * (x + 1)],
                                   in_=m4_ps)
                bds = []
                for x in range(2):
                    bd = small_pool.tile([128, 128], F16, tag="bd",
                                         name=f"bd_{p}_{x}")
                    sl = m4_sb[:, 32 * x:32 * (x + 1)]
                    m4_bcast = bass.AP(
                        tensor=sl.tensor,
                        offset=sl.offset,
                        ap=[list(sl.ap[0]), [0, 4], list(sl.ap[1])],
                    )
                    nc.gpsimd.tensor_mul(
                        out=bd.rearrange("p (r q) -> p r q", r=4),
                        in0=m4_bcast,
                        in1=blkmask.rearrange("p (r q) -> p r q", r=4),
                    )
                    bds.append(bd)
                return bds

            def emit_out(b, xt, bd, split_store=False):
                """4 block-diagonal matmuls against resident x^T + store.
                Stores ride the sync ring: a dma_start blocks its issuing
                engine's queue when the HWDGE ring backs up, and sync has
                nothing else to do after the upfront loads (scalar carries
                PE-critical copies).  split_store stores each half as soon
                as it's cast -- used for the last example to shorten the
                tail."""
                osb = osb_pool.tile([128, 2048], F16, tag="osb",
                                    name=f"osb_{b}")
                dst = out_t[b].rearrange("j q e -> (j q) e")
                for g in range(4):
                    o_ps = o_psum.tile([128, 512], F32, tag="o")
                    nc.tensor.matmul(
                        o_ps, lhsT=bd, rhs=xt[:, 512 * g:512 * (g + 1)],
                    )
                    if g % 2 == 0:
                        nc.scalar.copy(
                            out=osb[:, 512 * g:512 * (g + 1)], in_=o_ps)
                    else:
                        nc.vector.tensor_copy(
                            out=osb[:, 512 * g:512 * (g + 1)], in_=o_ps)
                    if split_store and g % 2 == 1:
                        nc.sync.dma_start(
                            out=dst[:, 512 * (g - 1):512 * (g + 1)],
                            in_=osb[:, 512 * (g - 1):512 * (g + 1)],
                        )
                if not split_store:
                    nc.sync.dma_start(out=dst, in_=osb)

            # All loads queued upfront; xn tiles first (gram needs them
            # immediately, xt only at the out stage).
            xns = [load_xn(b) for b in range(n_ex)]
            xts = [load_xt(b) for b in range(n_ex)]

            # Interleaved pair-pipelined schedule: gram blocks (PE-heavy)
            # alternate with chain/M-prep/output blocks (ACT/DVE-heavy) all
            # the way through, so the PE stream stays dense (HAM stays
            # warm) and the PSUM->SBUF casts overlap gram matmuls.  Every
            # consumer sits >=1 PE block after its producer.
            assert n_ex == 8
            sched = ["G0", "G1", "G2", "G3", "C0", "G4", "C1", "G5", "M0",
                     "G6", "C2", "G7", "O0", "M1", "O1", "C3", "O2", "M2",
                     "O3", "O4", "M3", "O5", "O6", "O7"]
            scs, bds = {}, {}
            for op in sched:
                kind, idx = op[0], int(op[1:])
                if kind == "G":
                    emit_gram(idx, xns[idx])
                elif kind == "C":
                    scs[idx] = emit_chain(idx)
                elif kind == "M":
                    bds[idx] = emit_m4bd(idx, scs.pop(idx))
                else:
                    emit_out(idx, xts[idx], bds[idx // 2][idx % 2],
                             split_store=(idx >= n_ex - 2))

    nc.compile()
    return nc


_CACHED_NC = None


def _get_nc():
    global _CACHED_NC
    if _CACHED_NC is None:
        _CACHED_NC = build_nc()
    return _CACHED_NC


def make_consts(wq, wk, wv):
    """c32 [128, 224] fp32: Wq tiled 4x down | blockdiag{Wk,Wk} | Wv^T
    tiled 4x along cols and 2x down.  c16 [128, 128] fp16: block mask."""
    c32 = np.zeros((128, 224), dtype=np.float32)
    c32[:, 0:32] = np.tile(wq, (4, 1))
    c32[0:D, 32:64] = wk
    c32[D:2 * D, 64:96] = wk
    c32[0:2 * D, 96:224] = np.tile(wv.T, (2, 4))
    blk = np.arange(128) // 32
    c16 = (blk[:, None] == blk[None, :]).astype(np.float16)
    return c32, c16


def make_xt(x16):
    """[n, 8192, 32] fp16 -> [n, 4, 32, 2048] with
    xt[b, j, k, e] = x[b, 2048 j + e, k]."""
    n = x16.shape[0]
    return np.ascontiguousarray(
        x16.reshape(n, 4, EB, D).transpose(0, 1, 3, 2)
    )


def make_xn(x16):
    """[n, 8192, 32] fp16 -> [n, 128, 2048] with
    xn[b, p, (t, j2, k)] = x[b, 2048 j2 + 128 t + p, k]."""
    n = x16.shape[0]
    return np.ascontiguousarray(
        x16.reshape(n, 4, 16, 128, D).transpose(0, 3, 2, 1, 4)
    ).reshape(n, 128, 2048)


def unpack_out(res_out):
    """[n, 4, 32, 2048] fp16 -> [n, 32, 8192] fp32."""
    return np.ascontiguousarray(
        res_out.astype(np.float32).transpose(0, 2, 1, 3)
    ).reshape(res_out.shape[0], D, S)


def make_in_maps(x, wq, wk, wv):
    c32, c16 = make_consts(wq, wk, wv)
    x16 = x.astype(np.float16)
    return [
        {
            "xt": make_xt(x16[c * PER_CORE:(c + 1) * PER_CORE]),
            "xn": make_xn(x16[c * PER_CORE:(c + 1) * PER_CORE]),
            "c32": c32,
            "c16": c16,
        }
        for c in range(N_CORES)
    ]


def kernel(x, Wq, Wk, Wv):
    x = np.asarray(x, dtype=np.float32)
    wq = np.asarray(Wq, dtype=np.float32).reshape(D, D)
    wk = np.asarray(Wk, dtype=np.float32).reshape(D, D)
    wv = np.asarray(Wv, dtype=np.float32).reshape(D, D)
    assert x.shape == (B, S, D)

    nc = _get_nc()
    in_maps = make_in_maps(x, wq, wk, wv)
    res = run_bass_kernel_spmd(nc, in_maps, list(range(N_CORES)))
    out = np.concatenate(
        [unpack_out(res.results[c]["out"]) for c in range(N_CORES)], axis=0
    )
    return out
